# revision 1
# baseline (speedup 1.0000x reference)
"""Trainium2 Bass kernel for relative-position multi-head attention.

Problem: B=8, N=1024, DIM=512, H=8, DH=64, MAX_POS=512
  out = softmax(q k^T * s + pos) v @ Wo + bo,  pos[i,r] = q_i . E[clip(i-r)+512] * s

Sharding: data-parallel over batch, one batch element per NeuronCore (8 cores).

Per-core algorithm (transposed layouts, bf16 matmuls, f32 PSUM accum):
  qT/kT  = (Wq*s)^T x^T, Wk^T x^T          (inner, N)
  va     = [x Wv | ones]                   (N, 65 per head) - PV lhsT + Z row
  Gxr[i,u] = q_i . E[clip(639-u)+512]      (reversed q.E table, per head)
     -> DRAM with row pitch 1281 so each banded 128-chunk of pos^T is a
        256B-aligned row: row(i, r_b) at idx = 10*i + 1 + i//128
  dma_gather(transpose=True): g[rr, q, i] = pos^T[r, i] for r_b = i_b-4+q
  S^T(r_b) psum = k_b^T q  + identity-matmul accumulate of g slices (banded)
  + ones x Gsat rank-1 terms (saturated |i-r| >= 512 ranges)
  exp on ScalarE (PSUM -> SBUF bf16); O^T accumulated with ones-augmented V
  (row 64 = softmax denominator Z); deferred normalize by 1/Z (approx recip);
  out^T = Wo^T O^T + bo.  Host transposes back.

The Gxr build + gather for head h+1 is emitted before attention of head h
(software pipelining) so the gather latency hides under attention matmuls.
"""

import numpy as np
import ml_dtypes
import sys

sys.path.insert(0, "/opt/trn_rl_repo")

import concourse.bass as bass  # noqa: E402
import concourse.mybir as mybir  # noqa: E402
import concourse.tile as tile  # noqa: E402
from concourse import bacc  # noqa: E402
from concourse.bass_utils import run_bass_kernel_spmd  # noqa: E402

B, N, DIM = 8, 1024, 512
H, DH = 8, 64
MAX_POS = 512
SCALE = DH ** -0.5
NB = N // 128          # 8 seq blocks
WGX = 1281             # padded Etxr width: [pad | 1279 cols | pad]
PITCH = 1281           # Gxr DRAM row pitch (elements)
NROWS = 10240          # gather-view rows of 128 elems (idx max 10238)
ESIZE = 9 * 128        # gather row: 9 blocks of 128

bf16 = ml_dtypes.bfloat16
BF = mybir.dt.bfloat16
F32 = mybir.dt.float32
I16 = mybir.dt.int16


def _ap(base, rel_off, pattern):
    """Custom access pattern relative to a tile's base AP."""
    b = base[:]
    return bass.AP(tensor=b.tensor, offset=b.offset + rel_off, ap=pattern)


def build_bass(wide_band=True, big_write=True, recip_mode="exact"):
    nc = bacc.Bacc()

    xT = nc.declare_dram_parameter("xT", [DIM, N], BF, isOutput=False)
    wq = nc.declare_dram_parameter("wq", [DIM, DIM], BF, isOutput=False)
    wk = nc.declare_dram_parameter("wk", [DIM, DIM], BF, isOutput=False)
    wv = nc.declare_dram_parameter("wv", [DIM, DIM], BF, isOutput=False)
    wo = nc.declare_dram_parameter("wo", [DIM, DIM], BF, isOutput=False)
    bo = nc.declare_dram_parameter("bo", [128, 4], F32, isOutput=False)
    etxr = nc.declare_dram_parameter("etxr", [128, WGX], BF, isOutput=False)
    et2 = nc.declare_dram_parameter("et2", [128, 33], BF, isOutput=False)
    iden = nc.declare_dram_parameter("iden", [128, 128], BF, isOutput=False)
    idxs = nc.declare_dram_parameter("idxs", [128, N // 16], I16, isOutput=False)
    out = nc.declare_dram_parameter("out", [DIM, N], F32, isOutput=True)

    with tile.TileContext(nc) as tc, tc.tile_pool(name="consts", bufs=1) as consts, \
            tc.tile_pool(name="qk", bufs=1) as qkpool, \
            tc.tile_pool(name="dram", bufs=2, space="DRAM") as drampool:

        # ---------- load constants ----------
        xT_sb = [consts.tile([128, N], BF, tag=f"xt{i}", name=f"xt{i}")
                 for i in range(4)]
        wq_sb = [consts.tile([128, DIM], BF, tag=f"wq{i}", name=f"wq{i}")
                 for i in range(4)]
        wk_sb = [consts.tile([128, DIM], BF, tag=f"wk{i}", name=f"wk{i}")
                 for i in range(4)]
        wv_sb = [consts.tile([128, DIM], BF, tag=f"wv{i}", name=f"wv{i}")
                 for i in range(4)]
        wo_sb = [consts.tile([128, DIM], BF, tag=f"wo{i}", name=f"wo{i}")
                 for i in range(4)]
        for k in range(4):
            sl = slice(128 * k, 128 * k + 128)
            nc.sync.dma_start(out=xT_sb[k], in_=xT[sl, :])
            nc.sync.dma_start(out=wq_sb[k], in_=wq[sl, :])
            nc.sync.dma_start(out=wk_sb[k], in_=wk[sl, :])
            nc.sync.dma_start(out=wv_sb[k], in_=wv[sl, :])
            nc.sync.dma_start(out=wo_sb[k], in_=wo[sl, :])
        etxr_sb = consts.tile([128, WGX], BF)
        nc.sync.dma_start(out=etxr_sb, in_=etxr[:, :])
        et2_sb = consts.tile([128, 33], BF)
        nc.sync.dma_start(out=et2_sb, in_=et2[:, :])
        iden_sb = consts.tile([128, 128], BF)
        nc.sync.dma_start(out=iden_sb, in_=iden[:, :])
        idxs_sb = consts.tile([128, N // 16], I16)
        nc.sync.dma_start(out=idxs_sb, in_=idxs[:, :])
        bo_sb = consts.tile([128, 4], F32)
        nc.sync.dma_start(out=bo_sb, in_=bo[:, :])
        ones_sb = consts.tile([1, 128], BF)
        nc.vector.memset(ones_sb, 1.0)

        # long-lived activations
        qT_sb = [qkpool.tile([128, N], BF, tag=f"qt{i}", name=f"qt{i}")
                 for i in range(4)]
        kT_sb = [qkpool.tile([128, N], BF, tag=f"kt{i}", name=f"kt{i}")
                 for i in range(4)]
        va_sb = [qkpool.tile([128, H * 65], BF, tag=f"va{i}", name=f"va{i}")
                 for i in range(NB)]
        oT_sb = [qkpool.tile([128, N], BF, tag=f"ot{i}", name=f"ot{i}")
                 for i in range(4)]

        # ---------- projections ----------
        with tc.tile_pool(name="proj_psum", bufs=2, space="PSUM") as pp:
            for m in range(4):
                for c in range(2):
                    csl = slice(512 * c, 512 * c + 512)
                    pq = pp.tile([128, 512], F32, tag="pq")
                    pk = pp.tile([128, 512], F32, tag="pk")
                    for k in range(4):
                        msl = slice(128 * m, 128 * m + 128)
                        nc.tensor.matmul(pq, wq_sb[k][:, msl], xT_sb[k][:, csl],
                                         start=(k == 0), stop=(k == 3))
                        nc.tensor.matmul(pk, wk_sb[k][:, msl], xT_sb[k][:, csl],
                                         start=(k == 0), stop=(k == 3))
                    nc.scalar.copy(out=qT_sb[m][:, csl], in_=pq)
                    nc.scalar.copy(out=kT_sb[m][:, csl], in_=pk)
            for nt in range(NB):
                pv = pp.tile([128, 512], F32, tag="pv")
                for k in range(4):
                    nsl = slice(128 * nt, 128 * nt + 128)
                    nc.tensor.matmul(pv, xT_sb[k][:, nsl], wv_sb[k],
                                     start=(k == 0), stop=(k == 3))
                vout = _ap(va_sb[nt], 0, [[H * 65, 128], [65, H], [1, 64]])
                vin = _ap(pv, 0, [[512, 128], [64, H], [1, 64]])
                nc.vector.tensor_copy(vout, vin)
                oc = _ap(va_sb[nt], 64, [[H * 65, 128], [65, H], [1, 1]])
                nc.vector.memset(oc, 1.0)

        # ---------- attention ----------
        with tc.tile_pool(name="gx_psum", bufs=2, space="PSUM") as gxp, \
                tc.tile_pool(name="st_psum", bufs=3, space="PSUM") as stp, \
                tc.tile_pool(name="o_psum", bufs=1, space="PSUM") as op, \
                tc.tile_pool(name="gs_psum", bufs=1, space="PSUM") as gsp, \
                tc.tile_pool(name="gxstage", bufs=2) as gxs, \
                tc.tile_pool(name="gpool", bufs=2) as gpool, \
                tc.tile_pool(name="es", bufs=3) as esp, \
                tc.tile_pool(name="small", bufs=2) as small:

            gsat = {}
            gtiles = {}
            zz = {}

            def emit_gx(h):
                """Gxr table -> DRAM -> transposing gather, plus Gsat rows."""
                pair, off = h // 2, 64 * (h % 2)
                hsl = slice(off, off + 64)
                qTh = qT_sb[pair]
                stg = gxs.tile([128, NB, WGX], BF, tag="stg", name=f"stg{h}")
                for ib in range(NB):
                    isl = slice(128 * ib, 128 * ib + 128)
                    c0 = 0
                    while c0 < WGX:
                        cw = min(512, WGX - c0)
                        pg = gxp.tile([128, 512], F32, tag="pg",
                                      name=f"pg{h}_{ib}_{c0}")
                        nc.tensor.matmul(pg[:, :cw], qTh[hsl, isl],
                                         etxr_sb[hsl, c0:c0 + cw],
                                         start=True, stop=True)
                        if (c0 // 512 + ib) % 2 == 0:
                            nc.scalar.copy(out=stg[:, ib, c0:c0 + cw],
                                           in_=pg[:, :cw])
                        else:
                            nc.vector.tensor_copy(stg[:, ib, c0:c0 + cw],
                                                  pg[:, :cw])
                        c0 += cw
                gxr = drampool.tile([N * PITCH], BF, name=f"gxr{h}")
                if big_write:
                    # one DMA per head: (partition, ib, elem) on both sides
                    dst = _ap(gxr, 0, [[PITCH, 128], [128 * PITCH, NB], [1, WGX]])
                    nc.sync.dma_start(out=dst, in_=stg[:])
                else:
                    for ib in range(NB):
                        dst = _ap(gxr, 128 * ib * PITCH, [[PITCH, 128], [1, WGX]])
                        nc.sync.dma_start(out=dst, in_=stg[:, ib, :])
                # Gsat rows: q.E[0] (row 0 / "lo"), q.E[1024] (row 1 / "hi")
                g0 = small.tile([1, N], BF, tag="gsat0", name=f"gsat0_{h}")
                g1 = small.tile([1, N], BF, tag="gsat1", name=f"gsat1_{h}")
                for c in range(2):
                    csl = slice(512 * c, 512 * c + 512)
                    # lhsT col 0 = E[0], col 32 = E[1024] (rest zero) so both
                    # result rows land on 32-aligned PSUM partitions.
                    ps = gsp.tile([33, 512], F32, tag="ps", name=f"ps{h}_{c}")
                    nc.tensor.matmul(ps, et2_sb[hsl, :], qTh[hsl, csl],
                                     start=True, stop=True)
                    nc.vector.tensor_copy(g0[:, csl], ps[0:1, :])
                    nc.vector.tensor_copy(g1[:, csl], ps[32:33, :])
                gsat[h] = (g0, g1)
                g = gpool.tile([128, 9, N], BF, tag="g", name=f"g{h}")
                src = _ap(gxr, 0, [[128, NROWS], [1, ESIZE]])
                nc.gpsimd.dma_gather(
                    out_ap=g[:], in_ap=src, idxs_ap=idxs_sb[:],
                    num_idxs=N, num_idxs_reg=N, elem_size=ESIZE, elem_step=128,
                    transpose=True, single_packet=False,
                )
                gtiles[h] = g

            emit_gx(0)
            for h in range(H):
                if h + 1 < H:
                    emit_gx(h + 1)
                pair, off = h // 2, 64 * (h % 2)
                hsl = slice(off, off + 64)
                qTh = qT_sb[pair]
                kTh = kT_sb[pair]
                g = gtiles.pop(h)
                g0, g1 = gsat.pop(h)

                oacc = [op.tile([65, 512], F32, tag=f"oacc{c}",
                                name=f"oacc{h}_{c}") for c in range(2)]
                for rb in range(NB):
                    rsl = slice(128 * rb, 128 * rb + 128)
                    ib_lo, ib_hi = max(0, rb - 4), min(NB, rb + 5)
                    lo_end = 128 * max(0, rb - 4)    # sat-low: i < lo_end
                    hi_st = 128 * min(NB, rb + 5)    # sat-high: i >= hi_st
                    es = esp.tile([128, N], BF, tag="es", name=f"es{h}_{rb}")
                    for c in range(2):
                        c0, c1 = 512 * c, 512 * c + 512
                        pst = stp.tile([128, 512], F32, tag="pst",
                                       name=f"pst{h}_{rb}_{c}")
                        # accumulation group: main k^T q first (start=True:
                        # clears bank, sets has_written), then one wide
                        # banded identity-matmul + sat rank-1 matmuls.
                        bb_lo = max(ib_lo * 128, c0)
                        bb_hi = min(ib_hi * 128, c1)
                        nbi0 = (bb_hi - bb_lo) // 128 if bb_lo < bb_hi else 0
                        nsub = ((1 if wide_band else nbi0) if nbi0 else 0) + \
                               (1 if lo_end > c0 else 0) + (1 if hi_st < c1 else 0)
                        nc.tensor.matmul(pst, kTh[hsl, rsl], qTh[hsl, c0:c1],
                                         start=True, stop=(nsub == 0))
                        j = 0
                        if bb_lo < bb_hi:
                            nbi = (bb_hi - bb_lo) // 128
                            ib0 = bb_lo // 128
                            q0 = rb - ib0 + 4
                            if wide_band:
                                rhs = _ap(g, q0 * N + bb_lo,
                                          [[9 * N, 128], [128 - N, nbi], [1, 128]])
                                j += 1
                                nc.tensor.matmul(pst[:, bb_lo - c0:bb_hi - c0],
                                                 iden_sb, rhs,
                                                 start=False, stop=(j == nsub))
                            else:
                                for t in range(nbi):
                                    a0 = bb_lo + 128 * t
                                    j += 1
                                    nc.tensor.matmul(
                                        pst[:, a0 - c0:a0 - c0 + 128], iden_sb,
                                        g[:, q0 - t, a0:a0 + 128],
                                        start=False, stop=(j == nsub))
                        if lo_end > c0:
                            a, bnd = c0, min(lo_end, c1)
                            j += 1
                            nc.tensor.matmul(pst[:, a - c0:bnd - c0], ones_sb,
                                             g0[:, a:bnd],
                                             start=False, stop=(j == nsub))
                        if hi_st < c1:
                            a, bnd = max(hi_st, c0), c1
                            j += 1
                            nc.tensor.matmul(pst[:, a - c0:bnd - c0], ones_sb,
                                             g1[:, a:bnd],
                                             start=False, stop=(j == nsub))
                        nc.scalar.activation(es[:, c0:c1], pst,
                                             mybir.ActivationFunctionType.Exp)
                        nc.tensor.matmul(oacc[c], va_sb[rb][:, 65 * h:65 * h + 65],
                                         es[:, c0:c1],
                                         start=(rb == 0), stop=(rb == NB - 1))

                # deferred normalization bookkeeping: stash Z rows + raw O^T
                for c in range(2):
                    csl = slice(512 * c, 512 * c + 512)
                    z = small.tile([1, 512], BF, tag=f"zz{h}_{c}",
                                   name=f"zz{h}_{c}")
                    nc.vector.tensor_copy(z, oacc[c][64:65, :])
                    zz[(h, c)] = z
                    nc.scalar.copy(out=oT_sb[pair][hsl, csl],
                                   in_=oacc[c][0:64, :])

            # ---------- normalization (batched, off the critical path) ----
            for h in range(H):
                pair, off = h // 2, 64 * (h % 2)
                hsl = slice(off, off + 64)
                for c in range(2):
                    csl = slice(512 * c, 512 * c + 512)
                    bz = gxp.tile([64, 512], F32, tag="pg", name=f"bz{h}_{c}")
                    nc.tensor.matmul(bz, ones_sb[:, :64], zz[(h, c)],
                                     start=True, stop=True)
                    rz = small.tile([128, 512], F32, tag="rz", name=f"rz{h}_{c}")
                    if recip_mode == "approx_psum":
                        nc.vector.reciprocal_approx_fast(out=rz[hsl, :], in_=bz)
                    elif recip_mode == "approx_sbuf":
                        zs = small.tile([128, 512], F32, tag="zs",
                                        name=f"zs{h}_{c}")
                        nc.scalar.copy(out=zs[hsl, :], in_=bz)
                        nc.vector.reciprocal_approx_fast(out=rz[hsl, :],
                                                         in_=zs[hsl, :])
                    else:
                        nc.vector.reciprocal(rz[hsl, :], bz)
                    nc.vector.tensor_mul(oT_sb[pair][hsl, csl],
                                         oT_sb[pair][hsl, csl], rz[hsl, :])

        # ---------- output projection ----------
        with tc.tile_pool(name="oproj_psum", bufs=4, space="PSUM") as opp, \
                tc.tile_pool(name="osb", bufs=4) as osb:
            for m in range(4):
                msl = slice(128 * m, 128 * m + 128)
                for c in range(2):
                    csl = slice(512 * c, 512 * c + 512)
                    po = opp.tile([128, 512], F32, tag="po")
                    for k in range(4):
                        nc.tensor.matmul(po, wo_sb[k][:, msl], oT_sb[k][:, csl],
                                         start=(k == 0), stop=(k == 3))
                    ot = osb.tile([128, 512], F32, tag="otf")
                    nc.scalar.add(out=ot, in_=po, add=bo_sb[:, m:m + 1])
                    nc.sync.dma_start(out=out[msl, csl], in_=ot)
    nc.compile()
    return nc


_NC_CACHE = {}


def _get_nc():
    if "nc" not in _NC_CACHE:
        _NC_CACHE["nc"] = build_bass()
    return _NC_CACHE["nc"]


def _host_prep(x, Wq, Wkv, Wo, bo, E):
    u = np.clip(639 - (np.arange(WGX) - 1), -512, 512) + 512
    etxr = E[u].T.astype(bf16)                                   # (64, WGX)
    etxr = np.concatenate([etxr, etxr], axis=0)                  # dup rows
    et2 = np.zeros((DH, 33), bf16)                               # (64, 33)
    et2[:, 0] = E[0].astype(bf16)
    et2[:, 32] = E[2 * MAX_POS].astype(bf16)
    et2 = np.concatenate([et2, et2], axis=0)
    ii = np.arange(N)
    idx = (10 * ii + 1 + ii // 128).astype(np.int16)
    idxs = np.zeros((16, N // 16), np.int16)
    idxs[ii % 16, ii // 16] = idx
    idxs = np.tile(idxs, (8, 1))                                 # (128, 64)
    common = {
        "wq": (Wq * SCALE).astype(bf16),
        "wk": Wkv[:, :DIM].astype(bf16),
        "wv": Wkv[:, DIM:].astype(bf16),
        "wo": Wo.astype(bf16),
        "bo": np.ascontiguousarray(bo.reshape(4, 128).T.astype(np.float32)),
        "etxr": np.ascontiguousarray(etxr),
        "et2": np.ascontiguousarray(et2),
        "iden": np.eye(128, dtype=bf16),
        "idxs": idxs,
    }
    in_maps = []
    for b in range(B):
        m = dict(common)
        m["xT"] = np.ascontiguousarray(x[b].T.astype(bf16))
        in_maps.append(m)
    return in_maps


def kernel(x, Wq, Wkv, Wo, bo, E):
    x, Wq, Wkv, Wo, bo, E = (np.asarray(a) for a in (x, Wq, Wkv, Wo, bo, E))
    nc = _get_nc()
    in_maps = _host_prep(x, Wq, Wkv, Wo, bo, E)
    res = run_bass_kernel_spmd(nc, in_maps, core_ids=list(range(B)))
    out = np.stack([np.asarray(res.results[b]["out"], dtype=np.float32).T
                    for b in range(B)])
    return out


if __name__ == "__main__":
    rng = np.random.default_rng(0)
    inputs = {
        "x": rng.standard_normal((B, N, DIM), dtype=np.float32),
        "Wq": rng.standard_normal((DIM, H * DH), dtype=np.float32) * DIM ** -0.5,
        "Wkv": rng.standard_normal((DIM, 2 * H * DH), dtype=np.float32) * DIM ** -0.5,
        "Wo": rng.standard_normal((H * DH, DIM), dtype=np.float32) * (H * DH) ** -0.5,
        "bo": np.zeros((DIM,), np.float32),
        "E": rng.standard_normal((2 * MAX_POS + 1, DH), dtype=np.float32),
    }
    o = kernel(**inputs)
    print("kernel ran, out shape", o.shape, "sample", o[0, 0, :4])



# revision 6
# speedup vs baseline: 1.1350x; 1.1350x over previous
"""Trainium2 Bass kernel for relative-position multi-head attention.

Problem: B=8, N=1024, DIM=512, H=8, DH=64, MAX_POS=512
  out = softmax(q k^T * s + pos) v @ Wo + bo,  pos[i,r] = q_i . E[clip(i-r)+512] * s

Sharding: data-parallel over batch, one batch element per NeuronCore (8 cores).

Per-core algorithm (transposed layouts, bf16 matmuls, f32 PSUM accum):
  qT/kT  = (Wq*s)^T x^T, Wk^T x^T          (inner, N)
  va     = [x Wv | ones]                   (N, 65 per head) - PV lhsT + Z row
  Gxr[i,u] = q_i . E[clip(639-u)+512]      (reversed q.E table, per head)
     -> DRAM with row pitch 1281 so each banded 128-chunk of pos^T is a
        256B-aligned row: row(i, r_b) at idx = 10*i + 1 + i//128
  dma_gather(transpose=True): g[rr, q, i] = pos^T[r, i] for r_b = i_b-4+q
  S^T(r_b) psum = k_b^T q  + identity-matmul accumulate of g slices (banded)
  + ones x Gsat rank-1 terms (saturated |i-r| >= 512 ranges)
  exp on ScalarE (PSUM -> SBUF bf16); O^T accumulated with ones-augmented V
  (row 64 = softmax denominator Z); per-head normalize: reciprocal of the
  single Z row, partition_broadcast, fused vector multiply PSUM->SBUF.
  out^T = Wo^T O^T + bo.  Host transposes back.

Pipelining: the Gxr build + DRAM write + gather for head h+2 is emitted at
the top of head h (2-deep), and the PV matmul for seq-block rb is emitted 2
blocks behind its S^T/exp so the Scalar-engine exp latency never stalls the
Tensor engine (keeps the PE array continuously busy so it ramps to 2.4GHz).
"""

import numpy as np
import ml_dtypes
import sys
from collections import deque

sys.path.insert(0, "/opt/trn_rl_repo")

import concourse.bass as bass  # noqa: E402
import concourse.mybir as mybir  # noqa: E402
import concourse.tile as tile  # noqa: E402
from concourse import bacc  # noqa: E402
from concourse.bass_utils import run_bass_kernel_spmd  # noqa: E402

B, N, DIM = 8, 1024, 512
H, DH = 8, 64
MAX_POS = 512
SCALE = DH ** -0.5
NB = N // 128          # 8 seq blocks
WGX = 1281             # padded Etxr width: [pad | 1279 cols | pad]
PITCH = 1281           # Gxr DRAM row pitch (elements)
NROWS = 10240          # gather-view rows of 128 elems (idx max 10238)
ESIZE = 9 * 128        # gather row: 9 blocks of 128

bf16 = ml_dtypes.bfloat16
BF = mybir.dt.bfloat16
F32 = mybir.dt.float32
I16 = mybir.dt.int16


def _ap(base, rel_off, pattern):
    """Custom access pattern relative to a tile's base AP."""
    b = base[:]
    return bass.AP(tensor=b.tensor, offset=b.offset + rel_off, ap=pattern)


def build_bass():
    nc = bacc.Bacc()

    xT = nc.declare_dram_parameter("xT", [DIM, N], BF, isOutput=False)
    wq = nc.declare_dram_parameter("wq", [DIM, DIM], BF, isOutput=False)
    wk = nc.declare_dram_parameter("wk", [DIM, DIM], BF, isOutput=False)
    wv = nc.declare_dram_parameter("wv", [DIM, DIM], BF, isOutput=False)
    wo = nc.declare_dram_parameter("wo", [DIM, DIM], BF, isOutput=False)
    bo = nc.declare_dram_parameter("bo", [128, 4], F32, isOutput=False)
    etxr = nc.declare_dram_parameter("etxr", [128, WGX], BF, isOutput=False)
    et2 = nc.declare_dram_parameter("et2", [128, 33], BF, isOutput=False)
    iden = nc.declare_dram_parameter("iden", [128, 128], BF, isOutput=False)
    idxs = nc.declare_dram_parameter("idxs", [128, N // 16], I16, isOutput=False)
    out = nc.declare_dram_parameter("out", [DIM, N], F32, isOutput=True)

    with tile.TileContext(nc) as tc, tc.tile_pool(name="consts", bufs=1) as consts, \
            tc.tile_pool(name="qk", bufs=1) as qkpool, \
            tc.tile_pool(name="dram", bufs=3, space="DRAM") as drampool:

        # ---------- load constants ----------
        xT_sb = [consts.tile([128, N], BF, tag=f"xt{i}", name=f"xt{i}")
                 for i in range(4)]
        wq_sb = [consts.tile([128, DIM], BF, tag=f"wq{i}", name=f"wq{i}")
                 for i in range(4)]
        wk_sb = [consts.tile([128, DIM], BF, tag=f"wk{i}", name=f"wk{i}")
                 for i in range(4)]
        wv_sb = [consts.tile([128, DIM], BF, tag=f"wv{i}", name=f"wv{i}")
                 for i in range(4)]
        wo_sb = [consts.tile([128, DIM], BF, tag=f"wo{i}", name=f"wo{i}")
                 for i in range(4)]
        for k in range(4):
            sl = slice(128 * k, 128 * k + 128)
            nc.sync.dma_start(out=xT_sb[k], in_=xT[sl, :])
            nc.sync.dma_start(out=wq_sb[k], in_=wq[sl, :])
            nc.sync.dma_start(out=wk_sb[k], in_=wk[sl, :])
            nc.sync.dma_start(out=wv_sb[k], in_=wv[sl, :])
            nc.sync.dma_start(out=wo_sb[k], in_=wo[sl, :])
        etxr_sb = consts.tile([128, WGX], BF)
        nc.sync.dma_start(out=etxr_sb, in_=etxr[:, :])
        et2_sb = consts.tile([128, 33], BF)
        nc.sync.dma_start(out=et2_sb, in_=et2[:, :])
        iden_sb = consts.tile([128, 128], BF)
        nc.sync.dma_start(out=iden_sb, in_=iden[:, :])
        idxs_sb = consts.tile([128, N // 16], I16)
        nc.sync.dma_start(out=idxs_sb, in_=idxs[:, :])
        bo_sb = consts.tile([128, 4], F32)
        nc.sync.dma_start(out=bo_sb, in_=bo[:, :])
        ones_sb = consts.tile([1, 128], BF)
        nc.vector.memset(ones_sb, 1.0)

        # long-lived activations
        qT_sb = [qkpool.tile([128, N], BF, tag=f"qt{i}", name=f"qt{i}")
                 for i in range(4)]
        kT_sb = [qkpool.tile([128, N], BF, tag=f"kt{i}", name=f"kt{i}")
                 for i in range(4)]
        va_sb = [qkpool.tile([128, H * 65], BF, tag=f"va{i}", name=f"va{i}")
                 for i in range(NB)]
        oT_sb = [qkpool.tile([128, N], BF, tag=f"ot{i}", name=f"ot{i}")
                 for i in range(4)]

        # ---------- q/k projections ----------
        with tc.tile_pool(name="proj_psum", bufs=2, space="PSUM") as pp:
            for m in range(4):
                for c in range(2):
                    csl = slice(512 * c, 512 * c + 512)
                    pq = pp.tile([128, 512], F32, tag="pq")
                    pk = pp.tile([128, 512], F32, tag="pk")
                    for k in range(4):
                        msl = slice(128 * m, 128 * m + 128)
                        nc.tensor.matmul(pq, wq_sb[k][:, msl], xT_sb[k][:, csl],
                                         start=(k == 0), stop=(k == 3))
                        nc.tensor.matmul(pk, wk_sb[k][:, msl], xT_sb[k][:, csl],
                                         start=(k == 0), stop=(k == 3))
                    nc.scalar.copy(out=qT_sb[m][:, csl], in_=pq)
                    nc.scalar.copy(out=kT_sb[m][:, csl], in_=pk)

        # ---------- attention ----------
        with tc.tile_pool(name="gx_psum", bufs=2, space="PSUM") as gxp, \
                tc.tile_pool(name="st_psum", bufs=4, space="PSUM") as stp, \
                tc.tile_pool(name="o_psum", bufs=1, space="PSUM") as op, \
                tc.tile_pool(name="gxstage", bufs=2) as gxs, \
                tc.tile_pool(name="gpool", bufs=3) as gpool, \
                tc.tile_pool(name="es", bufs=5) as esp, \
                tc.tile_pool(name="small", bufs=2) as small:

            gsat = {}
            gtiles = {}
            oaccs = {}
            es_tiles = {}
            pvq = deque()

            def emit_gx(h):
                """Gxr table -> DRAM -> transposing gather, plus Gsat rows."""
                pair, off = h // 2, 64 * (h % 2)
                hsl = slice(off, off + 64)
                qTh = qT_sb[pair]
                stg = gxs.tile([128, NB, WGX], BF, tag="stg", name=f"stg{h}")
                ci = 0
                for ib in range(NB):
                    isl = slice(128 * ib, 128 * ib + 128)
                    c0 = 0
                    while c0 < WGX:
                        cw = min(512, WGX - c0)
                        pg = gxp.tile([128, 512], F32, tag="pg",
                                      name=f"pg{h}_{ib}_{c0}")
                        nc.tensor.matmul(pg[:, :cw], qTh[hsl, isl],
                                         etxr_sb[hsl, c0:c0 + cw],
                                         start=True, stop=True)
                        # spread the PSUM->SBUF bf16 staging casts between
                        # scalar (which also runs exps) and vector (which
                        # also runs normalize); gpsimd cannot read PSUM
                        if ci % 5 == 0:
                            nc.scalar.copy(out=stg[:, ib, c0:c0 + cw],
                                           in_=pg[:, :cw])
                        else:
                            nc.vector.tensor_copy(stg[:, ib, c0:c0 + cw],
                                                  pg[:, :cw])
                        ci += 1
                        c0 += cw
                gxr = drampool.tile([N * PITCH], BF, name=f"gxr{h}")
                # one DMA per head: (partition, ib, elem) on both sides
                dst = _ap(gxr, 0, [[PITCH, 128], [128 * PITCH, NB], [1, WGX]])
                nc.sync.dma_start(out=dst, in_=stg[:])
                # Gsat rows: q.E[0] (row 0 / "lo"), q.E[1024] (row 1 / "hi")
                g0 = small.tile([1, N], BF, tag="gsat0", name=f"gsat0_{h}",
                                bufs=3)
                g1 = small.tile([1, N], BF, tag="gsat1", name=f"gsat1_{h}",
                                bufs=3)
                for c in range(2):
                    csl = slice(512 * c, 512 * c + 512)
                    # lhsT col 0 = E[0], col 32 = E[1024] (rest zero) so both
                    # result rows land on 32-aligned PSUM partitions.
                    ps = gxp.tile([128, 512], F32, tag="pg", name=f"ps{h}_{c}")
                    nc.tensor.matmul(ps[:33, :], et2_sb[hsl, :], qTh[hsl, csl],
                                     start=True, stop=True)
                    nc.vector.tensor_copy(g0[:, csl], ps[0:1, :])
                    nc.vector.tensor_copy(g1[:, csl], ps[32:33, :])
                gsat[h] = (g0, g1)
                g = gpool.tile([128, 9, N], BF, tag="g", name=f"g{h}")
                src = _ap(gxr, 0, [[128, NROWS], [1, ESIZE]])
                nc.gpsimd.dma_gather(
                    out_ap=g[:], in_ap=src, idxs_ap=idxs_sb[:],
                    num_idxs=N, num_idxs_reg=N, elem_size=ESIZE, elem_step=128,
                    transpose=True, single_packet=False,
                )
                gtiles[h] = g

            def emit_norm(h):
                """oT[head rows] = oacc rows / Z: reciprocal of the 1-row Z
                (cheap: DVE cost scales with free size, not partitions),
                ones-matmul broadcast to 64 partitions, then multiply."""
                pair, off = h // 2, 64 * (h % 2)
                hsl = slice(off, off + 64)
                for c in range(2):
                    csl = slice(512 * c, 512 * c + 512)
                    zf = small.tile([1, 512], F32, tag="zf", name=f"zf{h}_{c}")
                    nc.vector.tensor_copy(zf, oaccs[h][c][64:65, :])
                    rz1 = small.tile([1, 512], F32, tag="rz1",
                                     name=f"rz1_{h}_{c}")
                    nc.vector.reciprocal_approx_fast(out=rz1, in_=zf)
                    rzb = small.tile([1, 512], BF, tag="rzbf",
                                     name=f"rzb_{h}_{c}")
                    nc.vector.tensor_copy(rzb, rz1)
                    bz = gxp.tile([128, 512], F32, tag="pg",
                                  name=f"bz{h}_{c}")
                    nc.tensor.matmul(bz[:64, :], ones_sb[:, :64], rzb,
                                     start=True, stop=True)
                    rzs = small.tile([128, 512], F32, tag="rzs",
                                      name=f"rzs{h}_{c}")
                    nc.vector.tensor_copy(rzs[hsl, :], bz[:64, :])
                    if c == 0:
                        nc.scalar.copy(out=oT_sb[pair][hsl, csl],
                                       in_=oaccs[h][c][0:64, :])
                    else:
                        nc.vector.tensor_copy(oT_sb[pair][hsl, csl],
                                              oaccs[h][c][0:64, :])
                    nc.vector.tensor_mul(oT_sb[pair][hsl, csl],
                                         oT_sb[pair][hsl, csl], rzs[hsl, :])
                del oaccs[h]

            def emit_pv(h, rb):
                es = es_tiles.pop((h, rb))
                if rb == 0:
                    oaccs[h] = [op.tile([65, 512], F32, tag=f"oacc{c}",
                                        name=f"oacc{h}_{c}") for c in range(2)]
                for c in range(2):
                    nc.tensor.matmul(oaccs[h][c],
                                     va_sb[rb][:, 65 * h:65 * h + 65],
                                     es[:, 512 * c:512 * c + 512],
                                     start=(rb == 0), stop=(rb == NB - 1))
                if rb == NB - 1:
                    emit_norm(h)

            def emit_attn_iter(h, rb):
                pair, off = h // 2, 64 * (h % 2)
                hsl = slice(off, off + 64)
                qTh = qT_sb[pair]
                kTh = kT_sb[pair]
                g = gtiles[h]
                g0, g1 = gsat[h]
                rsl = slice(128 * rb, 128 * rb + 128)
                ib_lo, ib_hi = max(0, rb - 4), min(NB, rb + 5)
                lo_end = 128 * max(0, rb - 4)    # sat-low: i < lo_end
                hi_st = 128 * min(NB, rb + 5)    # sat-high: i >= hi_st
                es = esp.tile([128, N], BF, tag="es", name=f"es{h}_{rb}")
                es_tiles[(h, rb)] = es
                for c in range(2):
                    c0, c1 = 512 * c, 512 * c + 512
                    pst = stp.tile([128, 512], F32, tag="pst",
                                   name=f"pst{h}_{rb}_{c}")
                    # accumulation group: main k^T q first (start=True:
                    # clears bank, sets has_written), then one wide
                    # banded identity-matmul + sat rank-1 matmuls.
                    bb_lo = max(ib_lo * 128, c0)
                    bb_hi = min(ib_hi * 128, c1)
                    nbi = (bb_hi - bb_lo) // 128 if bb_lo < bb_hi else 0
                    nsub = (1 if nbi else 0) + \
                           (1 if lo_end > c0 else 0) + (1 if hi_st < c1 else 0)
                    nc.tensor.matmul(pst, kTh[hsl, rsl], qTh[hsl, c0:c1],
                                     start=True, stop=(nsub == 0))
                    j = 0
                    if nbi:
                        ib0 = bb_lo // 128
                        q0 = rb - ib0 + 4
                        rhs = _ap(g, q0 * N + bb_lo,
                                  [[9 * N, 128], [128 - N, nbi], [1, 128]])
                        j += 1
                        nc.tensor.matmul(pst[:, bb_lo - c0:bb_hi - c0],
                                         iden_sb, rhs,
                                         start=False, stop=(j == nsub))
                    if lo_end > c0:
                        a, bnd = c0, min(lo_end, c1)
                        j += 1
                        nc.tensor.matmul(pst[:, a - c0:bnd - c0], ones_sb,
                                         g0[:, a:bnd],
                                         start=False, stop=(j == nsub))
                    if hi_st < c1:
                        a, bnd = max(hi_st, c0), c1
                        j += 1
                        nc.tensor.matmul(pst[:, a - c0:bnd - c0], ones_sb,
                                         g1[:, a:bnd],
                                         start=False, stop=(j == nsub))
                    nc.scalar.activation(es[:, c0:c1], pst,
                                         mybir.ActivationFunctionType.Exp)
                pvq.append((h, rb))
                while len(pvq) > 2:
                    emit_pv(*pvq.popleft())

            emit_gx(0)

            # ---------- v projection (PSUM banks shared with pst tag) ----
            for nt in range(NB):
                pv = stp.tile([128, 512], F32, tag="pst", name=f"pv{nt}")
                for k in range(4):
                    nsl = slice(128 * nt, 128 * nt + 128)
                    nc.tensor.matmul(pv, xT_sb[k][:, nsl], wv_sb[k],
                                     start=(k == 0), stop=(k == 3))
                vout = _ap(va_sb[nt], 0, [[H * 65, 128], [65, H], [1, 64]])
                vin = _ap(pv, 0, [[512, 128], [64, H], [1, 64]])
                nc.vector.tensor_copy(vout, vin)
                oc = _ap(va_sb[nt], 64, [[H * 65, 128], [65, H], [1, 1]])
                nc.vector.memset(oc, 1.0)

            emit_gx(1)
            for h in range(H):
                if h + 2 < H:
                    emit_gx(h + 2)
                for rb in range(NB):
                    emit_attn_iter(h, rb)
                gtiles.pop(h)
                gsat.pop(h)
            while pvq:
                emit_pv(*pvq.popleft())

        # ---------- output projection ----------
        with tc.tile_pool(name="oproj_psum", bufs=4, space="PSUM") as opp, \
                tc.tile_pool(name="osb", bufs=4) as osb:
            for m in range(4):
                msl = slice(128 * m, 128 * m + 128)
                for c in range(2):
                    csl = slice(512 * c, 512 * c + 512)
                    po = opp.tile([128, 512], F32, tag="po")
                    for k in range(4):
                        nc.tensor.matmul(po, wo_sb[k][:, msl], oT_sb[k][:, csl],
                                         start=(k == 0), stop=(k == 3))
                    ot = osb.tile([128, 512], F32, tag="otf")
                    nc.scalar.add(out=ot, in_=po, add=bo_sb[:, m:m + 1])
                    nc.sync.dma_start(out=out[msl, csl], in_=ot)
    nc.compile()
    return nc


_NC_CACHE = {}


def _get_nc():
    if "nc" not in _NC_CACHE:
        _NC_CACHE["nc"] = build_bass()
    return _NC_CACHE["nc"]


def _host_prep(x, Wq, Wkv, Wo, bo, E):
    u = np.clip(639 - (np.arange(WGX) - 1), -512, 512) + 512
    etxr = E[u].T.astype(bf16)                                   # (64, WGX)
    etxr = np.concatenate([etxr, etxr], axis=0)                  # dup rows
    et2 = np.zeros((DH, 33), bf16)                               # (64, 33)
    et2[:, 0] = E[0].astype(bf16)
    et2[:, 32] = E[2 * MAX_POS].astype(bf16)
    et2 = np.concatenate([et2, et2], axis=0)
    ii = np.arange(N)
    idx = (10 * ii + 1 + ii // 128).astype(np.int16)
    idxs = np.zeros((16, N // 16), np.int16)
    idxs[ii % 16, ii // 16] = idx
    idxs = np.tile(idxs, (8, 1))                                 # (128, 64)
    common = {
        "wq": (Wq * SCALE).astype(bf16),
        "wk": Wkv[:, :DIM].astype(bf16),
        "wv": Wkv[:, DIM:].astype(bf16),
        "wo": Wo.astype(bf16),
        "bo": np.ascontiguousarray(bo.reshape(4, 128).T.astype(np.float32)),
        "etxr": np.ascontiguousarray(etxr),
        "et2": np.ascontiguousarray(et2),
        "iden": np.eye(128, dtype=bf16),
        "idxs": idxs,
    }
    in_maps = []
    for b in range(B):
        m = dict(common)
        m["xT"] = np.ascontiguousarray(x[b].T.astype(bf16))
        in_maps.append(m)
    return in_maps


def kernel(x, Wq, Wkv, Wo, bo, E):
    x, Wq, Wkv, Wo, bo, E = (np.asarray(a) for a in (x, Wq, Wkv, Wo, bo, E))
    nc = _get_nc()
    in_maps = _host_prep(x, Wq, Wkv, Wo, bo, E)
    res = run_bass_kernel_spmd(nc, in_maps, core_ids=list(range(B)))
    out = np.stack([np.asarray(res.results[b]["out"], dtype=np.float32).T
                    for b in range(B)])
    return out


if __name__ == "__main__":
    rng = np.random.default_rng(0)
    inputs = {
        "x": rng.standard_normal((B, N, DIM), dtype=np.float32),
        "Wq": rng.standard_normal((DIM, H * DH), dtype=np.float32) * DIM ** -0.5,
        "Wkv": rng.standard_normal((DIM, 2 * H * DH), dtype=np.float32) * DIM ** -0.5,
        "Wo": rng.standard_normal((H * DH, DIM), dtype=np.float32) * (H * DH) ** -0.5,
        "bo": np.zeros((DIM,), np.float32),
        "E": rng.standard_normal((2 * MAX_POS + 1, DH), dtype=np.float32),
    }
    o = kernel(**inputs)
    print("kernel ran, out shape", o.shape, "sample", o[0, 0, :4])
